# revision 15
# baseline (speedup 1.0000x reference)
"""Trainium2 Bass kernel for batched tanh-RNN (B=5000, T=8, V=5264, H=200).

  xh   = X @ W_ih.T + b_ih + b_hh          # [B, T, H]  (bulk of FLOPs)
  h_t  = tanh(xh[:, t] + h_{t-1} @ W_hh.T) # 8 steps
  out  = h_T @ W_fc.T + b_fc               # [B, V]

Data-parallel over batch across 8 NeuronCores (625 rows each), weights
replicated.  Everything on-core runs in "transposed" layout (hidden dim on
partitions, batch*time on the free dim):

  phase 1: xh.T[h, t*640+b] accumulated in PSUM over 42 v-tiles of 128.
           Mixed precision: timesteps t=0..5 use fp8(e4m3) X and W_ih with
           perf_mode=DoubleRow (2 k-tiles contracted per instruction, ~1.8x
           PE rate, half the HBM bytes); t=6,7 stay bf16 (their quantization
           error would reach the output un-attenuated; fp8 there pushes
           rel_absmax to ~0.02 = the limit, measured 0.0070 with this split
           vs 0.0047 all-bf16).  X is laid out host-side chunk-contiguous
           ([128, chunk, k, col]) so every DMA slab is one fully contiguous
           per-partition run.
  phase 2: h.T = tanh(W_hh.T.T @ h.T + xh_t.T); xh_t enters the PSUM
           accumulation group via an identity-stationary matmul, then one
           ACT Tanh PSUM->SBUF per tile.
  phase 3: out[b, v] = h.T-as-stationary @ W_fc.T tiles; b_fc is folded in
           as an extra contraction row (h8_b row 72 = 1.0, WFCB row 72 =
           b_fc), so no separate bias matmul.  Output is stored per
           (b-tile, v-chunk) from a small staging pool, alternating the two
           HWDGE rings, so the store tail is one chunk (~1us) instead of a
           full 2.7MB b-tile.
"""

import numpy as np

import concourse.bass as bass
import concourse.mybir as mybir
from concourse import bacc
from concourse.bass_utils import run_bass_kernel_spmd
from concourse.tile import TileContext

NCORE = 8
B, T, V, H = 5000, 8, 5264, 200
Bc = B // NCORE            # 625 batch rows per core
Bp = 640                   # padded batch per core (t-major col = t*Bp + b)
BT = Bp * T                # 5120
VP = 5376                  # V padded to 42*128
KT = VP // 128             # 42 contraction tiles
HA, HB = 128, H - 128      # hidden split across partition tiles (128 + 72)
HS = 208                   # W_ih fp8 free-dim stride (DoubleRow wants %16==0)

NT8 = 6                    # timesteps with fp8 X/W_ih
F8COLS = NT8 * Bp          # 3840 fp8 columns
F8CH = 480                 # fp8 phase-1 chunk width
NF8 = F8COLS // F8CH       # 8 fp8 chunks
B16CHUNKS = [(3840, 0, 512), (4352, 1, 512), (4864, 2, 256)]  # (col, idx, width)
SUB = 14                   # bf16 k-subslabs per chunk (42 = 3*14)

F32 = mybir.dt.float32
F32R = mybir.dt.float32r
BF16 = mybir.dt.bfloat16
F8E4 = mybir.dt.float8e4
AF = mybir.ActivationFunctionType
DROW = mybir.MatmulPerfMode.DoubleRow

# recurrence b-chunks (even, >=256 so float32r runs 1 cycle/row)
REC_CHUNKS = [(0, 320), (320, 320)]
# FC output v-chunks (all >=256, <=512)
FC_CHUNKS = [(i * 480, 480) for i in range(10)] + [(4800, 464)]
# FC batch tiles over padded 640 (stationary free dim = 128; the last tile
# computes 15 pad rows that are simply not stored)
FC_BTILES = [(0, 128, 128), (128, 128, 128), (256, 128, 128),
             (384, 128, 128), (512, 128, 113)]

_CACHE = {}
LAST_RESULT = None  # BassKernelResults of the most recent run (for test.py)


def _build(reps=1, bench_internal=False, phases=3):
    # Bacc (not raw Bass): its finalize() runs move_matmul_waits_to_ldweights
    # + generate_event_semaphores, required on TRN2 (max 1 sync wait/inst).
    # reps>1 re-emits the whole body (idempotent) for slope-based HW timing.
    # bench_internal keeps the big inputs as Internal DRAM (no upload per
    # call; contents garbage — timing is data-independent).
    nc = bacc.Bacc()

    def dram(name, shape, dt):
        if bench_internal:
            return nc.dram_tensor(name, shape, dt)
        return nc.declare_dram_parameter(name, shape, dt, isOutput=False)

    XT8 = dram("XT8", [128, NF8, KT, F8CH], F8E4)
    XT16 = dram("XT16", [128, 2, KT, 512], BF16)
    XT16C = dram("XT16C", [128, KT, 256], BF16)
    H0T = dram("H0T", [H, Bp], F32R)
    WIH8 = dram("WIH8", [128, KT, HS], F8E4)
    WIH16 = dram("WIH16", [128, KT, H], BF16)
    WHH = dram("WHH", [H, H], F32R)
    BIASH = dram("BIASH", [H, 1], F32)
    WFCA = dram("WFCA", [HA, V], BF16)
    WFCB = dram("WFCB", [HB + 1, V], BF16)
    IDEN = nc.declare_dram_parameter("IDEN", [128, 128], F32R, isOutput=False)
    YOUT = nc.declare_dram_parameter("YOUT", [Bc, V], BF16, isOutput=True)

    with TileContext(nc) as tc:
      for _rep in range(reps):
        with tc.tile_pool(name="const", bufs=1) as cpool, \
             tc.tile_pool(name="hpool", bufs=2) as hpool:
            # phase-1-critical weights first, on the scalar HWDGE ring (the
            # sync ring carries the X slabs).  All other preloads also go on
            # the scalar ring, but *interleaved* into program order behind
            # phase-1 activations: the ACT sequencer only reaches them after
            # the corresponding chunk completes, so they don't steal DMA
            # bandwidth from the (DMA-bound) fp8 X stream at the start.
            wih8_sb = cpool.tile([128, KT, HS], F8E4, tag="wih8")
            wih16_sb = cpool.tile([128, KT, H], BF16, tag="wih16")
            biash_a = cpool.tile([HA, 1], F32, tag="biash_a")
            biash_b = cpool.tile([HB, 1], F32, tag="biash_b")
            # bf16 phase runs first: its W_ih is the only startup-critical
            # load (split by k so compute starts after the first piece).
            for kp in range(3):
                nc.scalar.dma_start(out=wih16_sb[:, kp * SUB:(kp + 1) * SUB, :],
                                    in_=WIH16[:, kp * SUB:(kp + 1) * SUB, :])
            nc.scalar.dma_start(out=biash_a, in_=BIASH[0:HA, :])
            nc.scalar.dma_start(out=biash_b, in_=BIASH[HA:H, :])

            whh_a = cpool.tile([HA, H], F32R, tag="whh_a")
            whh_b = cpool.tile([HB, H], F32R, tag="whh_b")
            iden = cpool.tile([128, 128], F32R, tag="iden")
            xh_a = cpool.tile([HA, BT], F32R, tag="xh_a")
            xh_b = cpool.tile([HB, BT], F32R, tag="xh_b")
            wfc_a = cpool.tile([HA, V], BF16, tag="wfc_a")
            wfc_b = cpool.tile([HB + 1, V], BF16, tag="wfc_b")
            h8_a = cpool.tile([HA, Bp], BF16, tag="h8a")
            h8_b = cpool.tile([HB + 1, Bp], BF16, tag="h8b")
            cur_a = [hpool.tile([HA, 320], F32R, tag=f"ha{i}", name=f"cur_a{i}")
                     for i in range(2)]
            cur_b = [hpool.tile([HB, 320], F32R, tag=f"hb{i}", name=f"cur_b{i}")
                     for i in range(2)]
            # FC bias row: engines can't address a lone partition 72, so
            # memset all 73 rows; rows 0..71 are overwritten by the h cast.
            nc.vector.memset(h8_b, 1.0)
            # secondary preloads, issued in program-order slots behind the
            # bf16 chunks' activations: they stream during the bf16 phase's
            # DMA slack (it's PE-bound) instead of delaying the start or the
            # DMA-bound fp8 phase.
            preloads = [
                [lambda: nc.scalar.dma_start(out=wih8_sb[:, 0:22, :],
                                             in_=WIH8[:, 0:22, :]),
                 lambda: nc.scalar.dma_start(out=wih8_sb[:, 22:KT, :],
                                             in_=WIH8[:, 22:KT, :])],
                [lambda: nc.scalar.dma_start(out=whh_a, in_=WHH[0:HA, :]),
                 lambda: nc.scalar.dma_start(out=whh_b, in_=WHH[HA:H, :]),
                 lambda: nc.scalar.dma_start(out=iden, in_=IDEN[:, :])],
                [lambda: nc.scalar.dma_start(out=cur_a[0], in_=H0T[0:HA, 0:320]),
                 lambda: nc.scalar.dma_start(out=cur_a[1], in_=H0T[0:HA, 320:Bp]),
                 lambda: nc.scalar.dma_start(out=cur_b[0], in_=H0T[HA:H, 0:320]),
                 lambda: nc.scalar.dma_start(out=cur_b[1], in_=H0T[HA:H, 320:Bp])],
            ]

            with tc.tile_pool(name="x8", bufs=4) as x8pool, \
                 tc.tile_pool(name="x16", bufs=4) as x16pool, \
                 tc.tile_pool(name="ps1", bufs=2, space="PSUM") as ps1, \
                 tc.tile_pool(name="ps2", bufs=2, space="PSUM") as ps2:
                if phases != 4:
                    # ---- phase 1b: bf16 chunks (t=6,7), PE-bound
                    for (col, ci, ch) in B16CHUNKS:
                        pa = ps1.tile([HA, 512], F32, tag="pa16")
                        pb = ps1.tile([HB, 512], F32, tag="pb16")
                        # chunk 0 streams in 6 slabs of 7 k-tiles so compute
                        # starts ~2.5us in; later chunks use 14-k slabs
                        sub = 7 if ci == 0 else SUB
                        for s in range(KT // sub):
                            xs = x16pool.tile([128, sub, ch], BF16, tag="xs16")
                            if ci < 2:
                                src = XT16[:, ci, s * sub:(s + 1) * sub, :]
                            else:
                                src = XT16C[:, s * sub:(s + 1) * sub, :]
                            nc.sync.dma_start(out=xs, in_=src)
                            for j in range(sub):
                                k = s * sub + j
                                st = (k == 0)
                                sp = (k == KT - 1)
                                nc.tensor.matmul(
                                    pa[:, 0:ch], wih16_sb[:, k, 0:HA],
                                    xs[:, j, :], start=st, stop=sp)
                                nc.tensor.matmul(
                                    pb[:, 0:ch], wih16_sb[:, k, HA:H],
                                    xs[:, j, :], start=st, stop=sp)
                        nc.scalar.activation(
                            xh_a[:, col:col + ch], pa[:, 0:ch], AF.Identity,
                            bias=biash_a)
                        nc.scalar.activation(
                            xh_b[:, col:col + ch], pb[:, 0:ch], AF.Identity,
                            bias=biash_b)
                        for pl in preloads[ci]:
                            pl()

                # ---- recurrence step emitter (phase 2, interleaved into
                # 1a).  h kept as per-chunk [h, b] tiles so step t+1's
                # chunk-c matmuls depend only on step t's chunk-c tanh.
                def step(t):
                    nonlocal cur_a, cur_b
                    new_a = [hpool.tile([HA, 320], F32R, tag=f"ha{i}",
                                        name=f"new_a{i}") for i in range(2)]
                    new_b = [hpool.tile([HB, 320], F32R, tag=f"hb{i}",
                                        name=f"new_b{i}") for i in range(2)]
                    for ci, (c0, cn) in enumerate(REC_CHUNKS):
                        p0 = ps2.tile([HA, 320], F32, tag="p0")
                        p1 = ps2.tile([HB, 320], F32, tag="p1")
                        col = t * Bp + c0
                        nc.tensor.matmul(
                            p0[:, 0:cn], whh_a[:, 0:HA], cur_a[ci][:, 0:cn],
                            start=True, stop=False)
                        nc.tensor.matmul(
                            p0[:, 0:cn], whh_b[:, 0:HA], cur_b[ci][:, 0:cn],
                            start=False, stop=False)
                        nc.tensor.matmul(
                            p0[:, 0:cn], iden, xh_a[:, col:col + cn],
                            start=False, stop=True)
                        nc.tensor.matmul(
                            p1[:, 0:cn], whh_a[:, HA:H], cur_a[ci][:, 0:cn],
                            start=True, stop=False)
                        nc.tensor.matmul(
                            p1[:, 0:cn], whh_b[:, HA:H], cur_b[ci][:, 0:cn],
                            start=False, stop=False)
                        nc.tensor.matmul(
                            p1[:, 0:cn], iden[0:HB, 0:HB], xh_b[:, col:col + cn],
                            start=False, stop=True)
                        nc.scalar.activation(
                            new_a[ci][:, 0:cn], p0[:, 0:cn], AF.Tanh)
                        nc.scalar.activation(
                            new_b[ci][:, 0:cn], p1[:, 0:cn], AF.Tanh)
                    cur_a, cur_b = new_a, new_b

                # step t's xh columns [640t, 640t+640) are complete after
                # fp8 chunk ceil(640(t+1)/480)-1
                step_after = {1: 0, 2: 1, 3: 2, 5: 3, 6: 4, 7: 5}

                if phases != 4:
                    # ---- phase 1a: fp8 DoubleRow chunks (t=0..5), DMA-bound
                    # (two half-slabs of 11+10 DoubleRow k-pairs per chunk);
                    # recurrence steps slot into the per-chunk PE slack.
                    for c in range(NF8):
                        pa = ps1.tile([HA, 512], F32, tag="pa16",
                                      name="pa8")[:, 0:F8CH]
                        pb = ps1.tile([HB, 512], F32, tag="pb16",
                                      name="pb8")[:, 0:F8CH]
                        for (k0, nk) in ((0, 22), (22, 20)):
                            xs = x8pool.tile([128, nk, F8CH], F8E4, tag="xs8")
                            nc.sync.dma_start(out=xs, in_=XT8[:, c, k0:k0 + nk])
                            for j in range(nk // 2):
                                k = k0 + 2 * j
                                st = (k == 0)
                                sp = (k == KT - 2)
                                nc.tensor.matmul(
                                    pa, wih8_sb[:, k:k + 2, 0:HA],
                                    xs[:, 2 * j:2 * j + 2, :],
                                    start=st, stop=sp, perf_mode=DROW)
                                nc.tensor.matmul(
                                    pb, wih8_sb[:, k:k + 2, HA:H],
                                    xs[:, 2 * j:2 * j + 2, :],
                                    start=st, stop=sp, perf_mode=DROW)
                        c0 = c * F8CH
                        nc.scalar.activation(
                            xh_a[:, c0:c0 + F8CH], pa, AF.Identity, bias=biash_a)
                        nc.scalar.activation(
                            xh_b[:, c0:c0 + F8CH], pb, AF.Identity, bias=biash_b)
                        if phases >= 2 and c in step_after:
                            step(step_after[c])

                # FC weights stream during the tail recurrence steps; split
                # v-wise so phase 3's first chunks only wait for the halves
                # they read.
                VH = 2640
                nc.scalar.dma_start(out=wfc_a[:, 0:VH], in_=WFCA[:, 0:VH])
                nc.scalar.dma_start(out=wfc_b[:, 0:VH], in_=WFCB[:, 0:VH])
                nc.scalar.dma_start(out=wfc_a[:, VH:V], in_=WFCA[:, VH:V])
                nc.scalar.dma_start(out=wfc_b[:, VH:V], in_=WFCB[:, VH:V])

                if phases >= 2:
                    step(6)
                    step(7)

            # ---- phase 3: out = h_last @ W_fc.T (+ b_fc via h8_b row 72)
            if phases < 3:
                # still touch YOUT so outputs exist (gpsimd casts f32r->f32)
                if phases == 1:
                    nc.gpsimd.dma_start(out=YOUT[0:HA, 0:512], in_=xh_a[:, 0:512])
                else:
                    nc.gpsimd.dma_start(out=YOUT[0:HA, 0:320], in_=cur_a[0])
                continue
            with tc.tile_pool(name="stg", bufs=2) as stpool, \
                 tc.tile_pool(name="ps3", bufs=4, space="PSUM") as ps3:
                # cast h_last to bf16: FC stationaries get the fast (FWL)
                # weight-load path instead of ~1.1us fp32 self-loads
                nc.vector.tensor_copy(h8_a[:, 0:320], cur_a[0])
                nc.vector.tensor_copy(h8_a[:, 320:Bp], cur_a[1])
                nc.vector.tensor_copy(h8_b[0:HB, 0:320], cur_b[0])
                nc.vector.tensor_copy(h8_b[0:HB, 320:Bp], cur_b[1])

                # stage a whole b-tile, then one fully-contiguous DRAM store
                # per tile: v-chunk-granular stores (1.9KB strips strided
                # 21KB) thrash HBM write pages — measured ~5x slower.
                for bi, (b0, bn, bs) in enumerate(FC_BTILES):
                    yt = stpool.tile([128, V], BF16, tag="yt")
                    for (v0, vn) in FC_CHUNKS:
                        pf = ps3.tile([128, 480], F32, tag="pf")
                        nc.tensor.matmul(
                            pf[0:bn, 0:vn], h8_a[:, b0:b0 + bn],
                            wfc_a[:, v0:v0 + vn], start=True, stop=False)
                        nc.tensor.matmul(
                            pf[0:bn, 0:vn], h8_b[:, b0:b0 + bn],
                            wfc_b[:, v0:v0 + vn], start=False, stop=True)
                        nc.vector.tensor_copy(yt[0:bn, v0:v0 + vn],
                                              pf[0:bn, 0:vn])
                    eng = nc.sync if bi % 2 == 0 else nc.scalar
                    eng.dma_start(out=YOUT[b0:b0 + bs, :], in_=yt[0:bs, :])

    nc.finalize()
    return nc


def _prep_host(X, h0, W_ih, W_hh, b_ih, b_hh, W_fc, b_fc):
    import ml_dtypes
    f = np.float32
    bf = ml_dtypes.bfloat16
    f8 = ml_dtypes.float8_e4m3

    X = np.asarray(X, f)
    # X.T columns in t-major order: srcp[core, v, t*Bp+b] (v,b zero-padded)
    srcp = np.zeros((NCORE, VP, T, Bp), f)
    srcp[:, :V, :, :Bc] = X.reshape(NCORE, Bc, T, V).transpose(0, 3, 2, 1)
    srcp = srcp.reshape(NCORE, KT, 128, BT)
    # fp8 region (cols 0:3840) chunk-contiguous: XT8[core,p,c,k,j]
    XT8r = np.ascontiguousarray(
        srcp[:, :, :, :F8COLS].astype(f8)
        .reshape(NCORE, KT, 128, NF8, F8CH).transpose(0, 2, 3, 1, 4))
    # bf16 region (cols 3840:5120): two 512 chunks + one 256 chunk
    x16 = srcp[:, :, :, F8COLS:].astype(bf)       # [core, k, p, 1280]
    XT16r = np.ascontiguousarray(
        x16[:, :, :, :1024].reshape(NCORE, KT, 128, 2, 512)
        .transpose(0, 2, 3, 1, 4))
    XT16Cr = np.ascontiguousarray(x16[:, :, :, 1024:].transpose(0, 2, 1, 3))
    del srcp, x16

    wih_t = np.zeros((VP, H), f)
    wih_t[:V] = np.asarray(W_ih, f).T                      # [v, h]
    wih8 = np.zeros((KT, 128, HS), f8)
    wih8[:, :, :H] = wih_t.reshape(KT, 128, H).astype(f8)
    WIH8r = np.ascontiguousarray(wih8.transpose(1, 0, 2))
    WIH16r = np.ascontiguousarray(
        wih_t.astype(bf).reshape(KT, 128, H).transpose(1, 0, 2))

    WHHt = np.ascontiguousarray(np.asarray(W_hh, f).T)     # [h_prev, h_new]
    BIASHv = (np.asarray(b_ih, f) + np.asarray(b_hh, f)).reshape(H, 1).copy()
    wfct = np.asarray(W_fc, f).T                           # [h, v]
    WFCAv = np.ascontiguousarray(wfct[:HA].astype(bf))
    WFCBv = np.zeros((HB + 1, V), bf)
    WFCBv[:HB] = wfct[HA:].astype(bf)
    WFCBv[HB] = np.asarray(b_fc, f).astype(bf)
    H0T = np.zeros((NCORE, H, Bp), f)
    H0T[:, :, :Bc] = np.asarray(h0, f).reshape(NCORE, Bc, H).transpose(0, 2, 1)
    IDENv = np.eye(128, dtype=f)

    in_maps = []
    for i in range(NCORE):
        in_maps.append({
            "XT8": XT8r[i], "XT16": XT16r[i], "XT16C": XT16Cr[i],
            "H0T": H0T[i], "WIH8": WIH8r, "WIH16": WIH16r, "WHH": WHHt,
            "BIASH": BIASHv, "WFCA": WFCAv, "WFCB": WFCBv, "IDEN": IDENv,
        })
    return in_maps


def kernel(X, h0, W_ih, W_hh, b_ih, b_hh, W_fc, b_fc):
    global LAST_RESULT
    in_maps = _prep_host(X, h0, W_ih, W_hh, b_ih, b_hh, W_fc, b_fc)
    if "nc" not in _CACHE:
        _CACHE["nc"] = _build()
    res = run_bass_kernel_spmd(_CACHE["nc"], in_maps, list(range(NCORE)))
    LAST_RESULT = res
    return np.concatenate([res.results[i]["YOUT"] for i in range(NCORE)],
                      axis=0).astype(np.float32)
